# revision 9
# baseline (speedup 1.0000x reference)
"""MAMGCN submodule kernel for Trainium2, 8-core data-parallel over batch.

Problem (per reference):
  B=16, N=1024, F=64, T=12, K=3, F_OUT=64
  S = softmax_axis1(Vs @ sigmoid(lhs @ rhs^T + bs))
  out = relu(sum_k (cheb_k * S)^T @ x @ Theta_k)

Sharding: batch B=16 split across 8 cores (2 batches/core). All weights
replicated. Each core runs an identical Bass program on its shard.

Layout strategy per core/batch (n = destination node index, m = source):
  - product/P/S/E/A tiles keep m (or i) on partitions, n on free dim.
  - The cheb contraction uses x'-as-stationary matmuls producing
    z'[(t,f), n] transposed; Theta applied via block-diag (128,128)
    stationary; final (t,o)->(o,t)+transpose via PE transpose, with the
    softmax denominator folded in as a per-partition scale on the final
    relu copy.
  - All matmuls run in float32r (full PE rate at free>=256, ~1e-4 rounding).
"""
import numpy as np

import concourse.bass as bass
import concourse.mybir as mybir
import concourse.tile as tile
from concourse import bacc
from concourse.bass_utils import run_bass_kernel_spmd
from concourse.masks import make_identity

F32 = mybir.dt.float32
F32R = mybir.dt.float32r
AL = mybir.AluOpType
AF = mybir.ActivationFunctionType
AX = mybir.AxisListType

B_PER_CORE = 2
N = 1024
F = 64
T = 12
K = 3
FO = 64
NT = N // 128          # 8 n-tiles (128 rows each)
NQ = 4                 # n processed in quarters
HW = N // NQ // 1      # 256 free-dim per quarter
NC_TILES_PER_Q = HW // 128  # 2 c-subtiles of 128 per quarter
TQ = (T * F) // 128    # 6 (t,f)-chunks of x' (each = 2 t-values x 64 f)


def _emit_batch(nc, tc, pools, cst, b, x_d, bs_d, cheb_d, out_d):
    """Emit one batch's pipeline."""
    (stream, bigp, pe_pool, res_pool, psA, psZ, dram_pool) = pools

    # ---- Stage A: load x, reorder to x', attention row features ----
    xprime = bigp.tile([128, NT, T, F], F32R, tag="xp")
    xw1T = stream.tile([F, N], F32R, tag="xw1T", bufs=1)
    rhsBT = stream.tile([T, N], F32R, tag="rhsBT", bufs=1)
    for mi in range(NT):
        xnat = stream.tile([128, F, T], F32, tag="xnat")
        nc.sync.dma_start(out=xnat[:], in_=x_d.ap()[b, mi * 128:(mi + 1) * 128])
        # x' reorder (f,t) -> (t,f), rounded to fp32r
        nc.vector.tensor_copy(xprime[:, mi], xnat[:].rearrange("p f t -> p t f"))
        # xw1[n,f] = sum_t x*W1
        tmp = stream.tile([128, F, T], F32, tag="tmp", bufs=2)
        nc.vector.tensor_mul(tmp[:], xnat[:], cst["w1rep"][:])
        xw1_t = stream.tile([128, F], F32, tag="xw1t")
        nc.vector.tensor_reduce(out=xw1_t[:], in_=tmp[:], op=AL.add, axis=AX.X)
        # rhsB[n,t] = sum_f W3*x
        tmp2 = stream.tile([128, T, F], F32, tag="tmp2", bufs=2)
        nc.vector.tensor_mul(tmp2[:], xprime[:, mi].bitcast(F32), cst["w3rep"][:])
        rhsb_t = stream.tile([128, T], F32, tag="rhsbt")
        nc.vector.tensor_reduce(out=rhsb_t[:], in_=tmp2[:], op=AL.add, axis=AX.X)
        # transpose both to contraction-on-partitions layout
        pst64 = psA.tile([F, 128], F32, tag="a")
        nc.tensor.transpose(pst64[:], xw1_t[:], cst["ident"][:])
        nc.vector.tensor_copy(xw1T[:, mi * 128:(mi + 1) * 128], pst64[:])
        pst12 = psA.tile([T, 128], F32, tag="a")
        nc.tensor.transpose(pst12[:], rhsb_t[:], cst["ident"][:])
        nc.vector.tensor_copy(rhsBT[:, mi * 128:(mi + 1) * 128], pst12[:])

    # ---- Stage B: lhs^T = W2^T @ xw1^T  (12, N) ----
    lhsT_sb = stream.tile([T, N], F32R, tag="lhsT", bufs=1)
    for h in range(2):
        ps_l = psA.tile([T, 512], F32, tag="a")
        nc.tensor.matmul(ps_l[:], cst["w2r"][:], xw1T[:, h * 512:(h + 1) * 512],
                         start=True, stop=True)
        nc.vector.tensor_copy(lhsT_sb[:, h * 512:(h + 1) * 512], ps_l[:])

    # ---- per n-quarter pipeline ----
    for nh in range(NQ):
        HS = slice(nh * HW, (nh + 1) * HW)
        # Stage C: product + bs -> sigmoid -> P
        P_q = pe_pool.tile([128, NT, HW], F32R, tag="P")
        for ii in range(NT):
            ps_p = psA.tile([128, HW], F32, tag="a")
            nc.tensor.matmul(ps_p[:], lhsT_sb[:, ii * 128:(ii + 1) * 128],
                             rhsBT[:, HS], start=True, stop=True)
            bs_t = stream.tile([128, HW], F32, tag="bst")
            nc.sync.dma_start(out=bs_t[:], in_=bs_d.ap()[ii * 128:(ii + 1) * 128, HS])
            sgin = stream.tile([128, HW], F32, tag="sgin")
            nc.vector.tensor_add(sgin[:], ps_p[:], bs_t[:])
            nc.scalar.activation(P_q[:, ii], sgin[:], AF.Sigmoid)
        # Stage D: S = VsT^T @ P ; E = exp(S); colsum via ones-matmul
        E_q = pe_pool.tile([128, NT, HW], F32R, tag="E")
        ps_cs = psA.tile([1, HW], F32, tag="a")
        for ii in range(NT):
            ps_s = psA.tile([128, HW], F32, tag="a")
            for pi in range(NT):
                nc.tensor.matmul(ps_s[:], cst["vsT"][:, pi, ii * 128:(ii + 1) * 128],
                                 P_q[:, pi], start=(pi == 0), stop=(pi == NT - 1))
            nc.scalar.activation(E_q[:, ii], ps_s[:], AF.Exp)
            nc.tensor.matmul(ps_cs[:], cst["ones_r"][:], E_q[:, ii],
                             start=(ii == 0), stop=(ii == NT - 1))
        # softmax denominator reciprocal, then scatter to partitions
        cs_row = stream.tile([1, HW], F32, tag="cs")
        nc.vector.tensor_copy(cs_row[:], ps_cs[:])
        rc_d = dram_pool.tile([HW], F32, tag="rcd", name="rc_d")
        nc.sync.dma_start(out=rc_d.rearrange("(a b) -> a b", a=1),
                          in_=cs_row[:])
        rc_sc = stream.tile([128, NC_TILES_PER_Q], F32, tag="rcsc")
        nc.sync.dma_start(out=rc_sc[:],
                          in_=rc_d.rearrange("(c p) -> p c", p=128))
        recip_sb = stream.tile([128, NC_TILES_PER_Q], F32, tag="recip")
        nc.vector.reciprocal(recip_sb[:], rc_sc[:])
        # Stage E: A_k = cheb_k * E
        A_q = pe_pool.tile([128, K, NT, HW], F32R, tag="A", bufs=1)
        for mi in range(NT):
            for k in range(K):
                cheb_t = stream.tile([128, HW], F32, tag="chebt")
                nc.sync.dma_start(
                    out=cheb_t[:],
                    in_=cheb_d.ap()[k, mi * 128:(mi + 1) * 128, HS])
                nc.vector.tensor_mul(A_q[:, k, mi], cheb_t[:],
                                     E_q[:, mi].bitcast(F32))
        # Stage F: z' = x'^T-chunks @ A ; Theta via block-diag; transpose out
        res_tiles = []
        for _c in range(NC_TILES_PER_Q):
            res_c = res_pool.tile([128, FO, T], F32, tag="res", name=f"res{_c}")
            res_tiles.append(res_c)
        for q in range(TQ):
            ps_z = psZ.tile([128, K, HW], F32, tag="z", padded_shape=[128, K, 512])
            for mi in range(NT):
                for k in range(K):
                    nc.tensor.matmul(ps_z[:, k], xprime[:, mi, 2 * q:2 * q + 2, :],
                                     A_q[:, k, mi],
                                     start=(mi == 0), stop=(mi == NT - 1))
            ps_o = psA.tile([128, HW], F32, tag="a")
            for k in range(K):
                zs = stream.tile([128, HW], F32R, tag="zs")
                nc.vector.tensor_copy(zs[:], ps_z[:, k])
                nc.tensor.matmul(ps_o[:], cst["thbd"][:, k, :], zs[:],
                                 start=(k == 0), stop=(k == K - 1))
            os_t = stream.tile([128, HW], F32, tag="os")
            nc.vector.tensor_copy(os_t[:], ps_o[:])
            for c in range(NC_TILES_PER_Q):
                ps_tr = psA.tile([128, 128], F32, tag="a")
                nc.tensor.transpose(ps_tr[:], os_t[:, c * 128:(c + 1) * 128],
                                    cst["ident"][:])
                nc.vector.tensor_scalar(
                    out=res_tiles[c][:, :, 2 * q:2 * q + 2],
                    in0=ps_tr[:].rearrange("p (dt o) -> p o dt", o=FO),
                    scalar1=recip_sb[:, c:c + 1],
                    scalar2=0.0,
                    op0=AL.mult,
                    op1=AL.max,
                )
        for c in range(NC_TILES_PER_Q):
            nt_i = nh * NC_TILES_PER_Q + c
            nc.sync.dma_start(
                out=out_d.ap()[b, nt_i * 128:(nt_i + 1) * 128],
                in_=res_tiles[c][:])


def build_nc(repeat=1):
    nc = bacc.Bacc("TRN2", target_bir_lowering=False, debug=False, num_devices=8)
    x_d = nc.dram_tensor("x", [B_PER_CORE, N, F, T], F32, kind="ExternalInput")
    w1_d = nc.dram_tensor("W1", [T], F32, kind="ExternalInput")
    w2_d = nc.dram_tensor("W2", [F, T], F32, kind="ExternalInput")
    w3_d = nc.dram_tensor("W3", [F], F32, kind="ExternalInput")
    bs_d = nc.dram_tensor("bs", [N, N], F32, kind="ExternalInput")
    vs_d = nc.dram_tensor("Vs", [N, N], F32, kind="ExternalInput")
    cheb_d = nc.dram_tensor("cheb", [K, N, N], F32, kind="ExternalInput")
    th_d = nc.dram_tensor("Theta", [K, F, FO], F32, kind="ExternalInput")
    out_d = nc.dram_tensor("out", [B_PER_CORE, N, FO, T], F32,
                           kind="ExternalOutput")

    with tile.TileContext(nc) as tc:
        with (
            tc.tile_pool(name="consts", bufs=1) as consts,
            tc.tile_pool(name="stream", bufs=3) as stream,
            tc.tile_pool(name="bigp", bufs=1) as bigp,
            tc.tile_pool(name="pe", bufs=2) as pe_pool,
            tc.tile_pool(name="res", bufs=4) as res_pool,
            tc.tile_pool(name="dram", bufs=2, space="DRAM") as dram_pool,
            tc.tile_pool(name="psA", bufs=4, space="PSUM") as psA,
            tc.tile_pool(name="psZ", bufs=1, space="PSUM") as psZ,
        ):
            cst = {}
            ident = consts.tile([128, 128], F32)
            make_identity(nc, ident[:])
            cst["ident"] = ident
            # ones vectors (fp32r via rounding copy)
            onesf = consts.tile([128, 1], F32)
            nc.vector.memset(onesf[:], 1.0)
            ones_r = consts.tile([128, 1], F32R)
            nc.vector.tensor_copy(ones_r[:], onesf[:])
            cst["ones_r"] = ones_r
            # broadcast W1 / W3 replicas
            w1rep = consts.tile([128, F, T], F32)
            nc.gpsimd.dma_start(
                out=w1rep[:],
                in_=bass.AP(tensor=w1_d, offset=0, ap=[[0, 128], [0, F], [1, T]]))
            cst["w1rep"] = w1rep
            w3rep = consts.tile([128, T, F], F32)
            nc.gpsimd.dma_start(
                out=w3rep[:],
                in_=bass.AP(tensor=w3_d, offset=0, ap=[[0, 128], [0, T], [1, F]]))
            cst["w3rep"] = w3rep
            # W2 (f, t) fp32r
            w2f = consts.tile([F, T], F32)
            nc.sync.dma_start(out=w2f[:], in_=w2_d.ap())
            w2r = consts.tile([F, T], F32R)
            nc.vector.tensor_copy(w2r[:], w2f[:])
            cst["w2r"] = w2r
            # block-diagonal Theta (128, K, 128)
            thbd_f = consts.tile([128, K, 128], F32)
            nc.vector.memset(thbd_f[:], 0.0)
            for k in range(K):
                nc.sync.dma_start(out=thbd_f[0:F, k, 0:FO], in_=th_d.ap()[k])
                nc.sync.dma_start(out=thbd_f[F:128, k, FO:128], in_=th_d.ap()[k])
            thbd = consts.tile([128, K, 128], F32R)
            nc.vector.tensor_copy(thbd[:], thbd_f[:])
            cst["thbd"] = thbd
            # VsT (p-partitioned Vs transpose), fp32r
            vsT = consts.tile([128, NT, N], F32R)
            for pi in range(NT):
                for ii in range(NT):
                    vtmp = stream.tile([128, 128], F32, tag="vtmp", bufs=2)
                    nc.sync.dma_start(
                        out=vtmp[:],
                        in_=vs_d.ap()[ii * 128:(ii + 1) * 128,
                                      pi * 128:(pi + 1) * 128])
                    ps_v = psA.tile([128, 128], F32, tag="a")
                    nc.tensor.transpose(ps_v[:], vtmp[:], ident[:])
                    nc.vector.tensor_copy(vsT[:, pi, ii * 128:(ii + 1) * 128],
                                          ps_v[:])
            cst["vsT"] = vsT

            pools = (stream, bigp, pe_pool, res_pool, psA, psZ, dram_pool)
            for _ in range(repeat):
                for b in range(B_PER_CORE):
                    _emit_batch(nc, tc, pools, cst, b, x_d, bs_d, cheb_d, out_d)
    nc.compile()
    return nc


_NC_CACHE = {}


def _get_nc(repeat=1):
    if repeat not in _NC_CACHE:
        _NC_CACHE[repeat] = build_nc(repeat)
    return _NC_CACHE[repeat]


def kernel(x, W1, W2, W3, bs, Vs, cheb, Theta, repeat=1):
    x = np.asarray(x, dtype=np.float32)
    W1 = np.asarray(W1, dtype=np.float32)
    W2 = np.asarray(W2, dtype=np.float32)
    W3 = np.asarray(W3, dtype=np.float32)
    bs = np.asarray(bs, dtype=np.float32).reshape(N, N)
    Vs = np.asarray(Vs, dtype=np.float32)
    cheb = np.asarray(cheb, dtype=np.float32)
    Theta = np.asarray(Theta, dtype=np.float32)

    nc = _get_nc(repeat)
    n_cores = 8
    in_maps = []
    for c in range(n_cores):
        in_maps.append({
            "x": x[c * B_PER_CORE:(c + 1) * B_PER_CORE],
            "W1": W1, "W2": W2, "W3": W3,
            "bs": bs, "Vs": Vs, "cheb": cheb, "Theta": Theta,
        })
    res = run_bass_kernel_spmd(nc, in_maps, list(range(n_cores)))
    return np.concatenate([res.results[c]["out"] for c in range(n_cores)], axis=0)


# revision 12
# speedup vs baseline: 439.1717x; 439.1717x over previous
"""MAMGCN submodule kernel for Trainium2, 8-core data-parallel over batch.

Problem (per reference):
  B=16, N=1024, F=64, T=12, K=3, F_OUT=64
  S = softmax_axis1(Vs @ sigmoid(lhs @ rhs^T + bs))
  out = relu(sum_k (cheb_k * S)^T @ x @ Theta_k)

Sharding: batch B=16 split across 8 cores (2 batches/core). All weights
replicated. Each core runs an identical Bass program on its shard.

Layout strategy per core/batch (n = destination node index, m = source):
  - product/P/S/E/A tiles keep m (or i) on partitions, n on free dim.
  - The cheb contraction uses x'-as-stationary matmuls producing
    z'[(t,f), n] transposed; Theta applied via block-diag (128,128)
    stationary; final (t,o)->(o,t)+transpose via PE transpose, with the
    softmax denominator folded in as a per-partition scale on the final
    relu copy.
  - All matmuls run in float32r (full PE rate at free>=256, ~1e-4 rounding).
"""
import numpy as np

import concourse.bass as bass
import concourse.mybir as mybir
import concourse.tile as tile
from concourse import bacc
from concourse.bass_utils import run_bass_kernel_spmd
from concourse.masks import make_identity

F32 = mybir.dt.float32
F32R = mybir.dt.float32r
AL = mybir.AluOpType
AF = mybir.ActivationFunctionType
AX = mybir.AxisListType

B_PER_CORE = 2
N = 1024
F = 64
T = 12
K = 3
FO = 64
NT = N // 128          # 8 n-tiles (128 rows each)
NQ = 4                 # n processed in quarters
HW = N // NQ // 1      # 256 free-dim per quarter
NC_TILES_PER_Q = HW // 128  # 2 c-subtiles of 128 per quarter
TQ = (T * F) // 128    # 6 (t,f)-chunks of x' (each = 2 t-values x 64 f)


def _emit_batch(nc, tc, pools, cst, b, x_d, bs_d, cheb_d, out_d):
    """Emit one batch's pipeline."""
    (stream, bigp, pe_pool, res_pool, psA, psZ, dram_pool) = pools

    # ---- Stage A: load x, reorder to x', attention row features ----
    xprime = bigp.tile([128, NT, T, F], F32R, tag="xp")
    xw1T = stream.tile([F, N], F32R, tag="xw1T", bufs=1)
    rhsBT = stream.tile([T, N], F32R, tag="rhsBT", bufs=1)
    for mi in range(NT):
        xnat = stream.tile([128, F, T], F32, tag="xnat")
        nc.sync.dma_start(out=xnat[:], in_=x_d.ap()[b, mi * 128:(mi + 1) * 128])
        # x' reorder (f,t) -> (t,f), rounded to fp32r
        nc.vector.tensor_copy(xprime[:, mi], xnat[:].rearrange("p f t -> p t f"))
        # xw1[n,f] = sum_t x*W1
        tmp = stream.tile([128, F, T], F32, tag="tmp", bufs=2)
        nc.vector.tensor_mul(tmp[:], xnat[:], cst["w1rep"][:])
        xw1_t = stream.tile([128, F], F32, tag="xw1t")
        nc.vector.tensor_reduce(out=xw1_t[:], in_=tmp[:], op=AL.add, axis=AX.X)
        # rhsB[n,t] = sum_f W3*x
        tmp2 = stream.tile([128, T, F], F32, tag="tmp2", bufs=2)
        nc.vector.tensor_mul(tmp2[:], xprime[:, mi].bitcast(F32), cst["w3rep"][:])
        rhsb_t = stream.tile([128, T], F32, tag="rhsbt")
        nc.vector.tensor_reduce(out=rhsb_t[:], in_=tmp2[:], op=AL.add, axis=AX.X)
        # transpose both to contraction-on-partitions layout
        pst64 = psA.tile([F, 128], F32, tag="a")
        nc.tensor.transpose(pst64[:], xw1_t[:], cst["ident"][:])
        nc.vector.tensor_copy(xw1T[:, mi * 128:(mi + 1) * 128], pst64[:])
        pst12 = psA.tile([T, 128], F32, tag="a")
        nc.tensor.transpose(pst12[:], rhsb_t[:], cst["ident"][:])
        nc.vector.tensor_copy(rhsBT[:, mi * 128:(mi + 1) * 128], pst12[:])

    # ---- Stage B: lhs^T = W2^T @ xw1^T  (12, N) ----
    lhsT_sb = stream.tile([T, N], F32R, tag="lhsT", bufs=1)
    for h in range(2):
        ps_l = psA.tile([T, 512], F32, tag="a")
        nc.tensor.matmul(ps_l[:], cst["w2r"][:], xw1T[:, h * 512:(h + 1) * 512],
                         start=True, stop=True)
        nc.vector.tensor_copy(lhsT_sb[:, h * 512:(h + 1) * 512], ps_l[:])

    # ---- per n-quarter pipeline ----
    for nh in range(NQ):
        HS = slice(nh * HW, (nh + 1) * HW)
        # Stage C: product + bs -> sigmoid -> P
        P_q = pe_pool.tile([128, NT, HW], F32R, tag="P")
        for ii in range(NT):
            ps_p = psA.tile([128, HW], F32, tag="a")
            nc.tensor.matmul(ps_p[:], lhsT_sb[:, ii * 128:(ii + 1) * 128],
                             rhsBT[:, HS], start=True, stop=True)
            bs_t = stream.tile([128, HW], F32, tag="bst")
            nc.sync.dma_start(out=bs_t[:], in_=bs_d.ap()[ii * 128:(ii + 1) * 128, HS])
            sgin = stream.tile([128, HW], F32, tag="sgin")
            nc.vector.tensor_add(sgin[:], ps_p[:], bs_t[:])
            nc.scalar.activation(P_q[:, ii], sgin[:], AF.Sigmoid)
        # Stage D: S = VsT^T @ P ; E = exp(S); colsum via ones-matmul
        E_q = pe_pool.tile([128, NT, HW], F32R, tag="E")
        ps_cs = psA.tile([1, HW], F32, tag="a")
        for ii in range(NT):
            ps_s = psA.tile([128, HW], F32, tag="a")
            for pi in range(NT):
                nc.tensor.matmul(ps_s[:], cst["vsT"][:, pi, ii * 128:(ii + 1) * 128],
                                 P_q[:, pi], start=(pi == 0), stop=(pi == NT - 1))
            nc.scalar.activation(E_q[:, ii], ps_s[:], AF.Exp)
            nc.tensor.matmul(ps_cs[:], cst["ones_r"][:], E_q[:, ii],
                             start=(ii == 0), stop=(ii == NT - 1))
        # softmax denominator reciprocal, then scatter to partitions
        cs_row = stream.tile([1, HW], F32, tag="cs")
        nc.vector.tensor_copy(cs_row[:], ps_cs[:])
        rc_d = dram_pool.tile([HW], F32, tag="rcd", name="rc_d")
        nc.sync.dma_start(out=rc_d.rearrange("(a b) -> a b", a=1),
                          in_=cs_row[:])
        rc_sc = stream.tile([128, NC_TILES_PER_Q], F32, tag="rcsc")
        nc.sync.dma_start(out=rc_sc[:],
                          in_=rc_d.rearrange("(c p) -> p c", p=128))
        recip_sb = stream.tile([128, NC_TILES_PER_Q], F32, tag="recip")
        nc.vector.reciprocal(recip_sb[:], rc_sc[:])
        # Stage E: A_k = cheb_k * E
        A_q = pe_pool.tile([128, K, NT, HW], F32R, tag="A", bufs=1)
        for mi in range(NT):
            for k in range(K):
                cheb_t = stream.tile([128, HW], F32, tag="chebt")
                nc.sync.dma_start(
                    out=cheb_t[:],
                    in_=cheb_d.ap()[k, mi * 128:(mi + 1) * 128, HS])
                nc.vector.tensor_mul(A_q[:, k, mi], cheb_t[:],
                                     E_q[:, mi].bitcast(F32))
        # Stage F: z' = x'^T-chunks @ A ; Theta via block-diag; transpose out
        res_tiles = []
        for _c in range(NC_TILES_PER_Q):
            res_c = res_pool.tile([128, FO, T], F32, tag="res", name=f"res{_c}")
            res_tiles.append(res_c)
        for q in range(TQ):
            ps_z = psZ.tile([128, K, HW], F32, tag="z", padded_shape=[128, K, 512])
            for mi in range(NT):
                for k in range(K):
                    nc.tensor.matmul(ps_z[:, k], xprime[:, mi, 2 * q:2 * q + 2, :],
                                     A_q[:, k, mi],
                                     start=(mi == 0), stop=(mi == NT - 1))
            ps_o = psA.tile([128, HW], F32, tag="a")
            for k in range(K):
                zs = stream.tile([128, HW], F32R, tag="zs")
                nc.vector.tensor_copy(zs[:], ps_z[:, k])
                nc.tensor.matmul(ps_o[:], cst["thbd"][:, k, :], zs[:],
                                 start=(k == 0), stop=(k == K - 1))
            os_t = stream.tile([128, HW], F32, tag="os")
            nc.vector.tensor_copy(os_t[:], ps_o[:])
            for c in range(NC_TILES_PER_Q):
                ps_tr = psA.tile([128, 128], F32, tag="a")
                nc.tensor.transpose(ps_tr[:], os_t[:, c * 128:(c + 1) * 128],
                                    cst["ident"][:])
                nc.vector.tensor_scalar(
                    out=res_tiles[c][:, :, 2 * q:2 * q + 2],
                    in0=ps_tr[:].rearrange("p (dt o) -> p o dt", o=FO),
                    scalar1=recip_sb[:, c:c + 1],
                    scalar2=0.0,
                    op0=AL.mult,
                    op1=AL.max,
                )
        for c in range(NC_TILES_PER_Q):
            nt_i = nh * NC_TILES_PER_Q + c
            nc.sync.dma_start(
                out=out_d.ap()[b, nt_i * 128:(nt_i + 1) * 128],
                in_=res_tiles[c][:])


def build_nc(repeat=1):
    nc = bacc.Bacc("TRN2", target_bir_lowering=False, debug=False, num_devices=8)
    x_d = nc.dram_tensor("x", [B_PER_CORE, N, F, T], F32, kind="ExternalInput")
    w1_d = nc.dram_tensor("W1", [T], F32, kind="ExternalInput")
    w2_d = nc.dram_tensor("W2", [F, T], F32, kind="ExternalInput")
    w3_d = nc.dram_tensor("W3", [F], F32, kind="ExternalInput")
    bs_d = nc.dram_tensor("bs", [N, N], F32, kind="ExternalInput")
    vs_d = nc.dram_tensor("Vs", [N, N], F32, kind="ExternalInput")
    cheb_d = nc.dram_tensor("cheb", [K, N, N], F32, kind="ExternalInput")
    th_d = nc.dram_tensor("Theta", [K, F, FO], F32, kind="ExternalInput")
    out_d = nc.dram_tensor("out", [B_PER_CORE, N, FO, T], F32,
                           kind="ExternalOutput")

    with tile.TileContext(nc) as tc:
        with (
            tc.tile_pool(name="consts", bufs=1) as consts,
            tc.tile_pool(name="stream", bufs=3) as stream,
            tc.tile_pool(name="bigp", bufs=1) as bigp,
            tc.tile_pool(name="pe", bufs=2) as pe_pool,
            tc.tile_pool(name="res", bufs=4) as res_pool,
            tc.tile_pool(name="dram", bufs=2, space="DRAM") as dram_pool,
            tc.tile_pool(name="psA", bufs=4, space="PSUM") as psA,
            tc.tile_pool(name="psZ", bufs=1, space="PSUM") as psZ,
        ):
            cst = {}
            ident = consts.tile([128, 128], F32)
            make_identity(nc, ident[:])
            cst["ident"] = ident
            # ones vectors (fp32r via rounding copy)
            onesf = consts.tile([128, 1], F32)
            nc.vector.memset(onesf[:], 1.0)
            ones_r = consts.tile([128, 1], F32R)
            nc.vector.tensor_copy(ones_r[:], onesf[:])
            cst["ones_r"] = ones_r
            # broadcast W1 / W3 replicas
            w1rep = consts.tile([128, F, T], F32)
            nc.gpsimd.dma_start(
                out=w1rep[:],
                in_=bass.AP(tensor=w1_d, offset=0, ap=[[0, 128], [0, F], [1, T]]))
            cst["w1rep"] = w1rep
            w3rep = consts.tile([128, T, F], F32)
            nc.gpsimd.dma_start(
                out=w3rep[:],
                in_=bass.AP(tensor=w3_d, offset=0, ap=[[0, 128], [0, T], [1, F]]))
            cst["w3rep"] = w3rep
            # W2 (f, t) fp32r
            w2f = consts.tile([F, T], F32)
            nc.sync.dma_start(out=w2f[:], in_=w2_d.ap())
            w2r = consts.tile([F, T], F32R)
            nc.vector.tensor_copy(w2r[:], w2f[:])
            cst["w2r"] = w2r
            # block-diagonal Theta (128, K, 128)
            thbd_f = consts.tile([128, K, 128], F32)
            nc.vector.memset(thbd_f[:], 0.0)
            for k in range(K):
                nc.sync.dma_start(out=thbd_f[0:F, k, 0:FO], in_=th_d.ap()[k])
                nc.sync.dma_start(out=thbd_f[F:128, k, FO:128], in_=th_d.ap()[k])
            thbd = consts.tile([128, K, 128], F32R)
            nc.vector.tensor_copy(thbd[:], thbd_f[:])
            cst["thbd"] = thbd
            # VsT (p-partitioned Vs transpose), fp32r
            vsT = consts.tile([128, NT, N], F32R)
            for pi in range(NT):
                for ii in range(NT):
                    vtmp = stream.tile([128, 128], F32, tag="vtmp", bufs=2)
                    nc.sync.dma_start(
                        out=vtmp[:],
                        in_=vs_d.ap()[ii * 128:(ii + 1) * 128,
                                      pi * 128:(pi + 1) * 128])
                    ps_v = psA.tile([128, 128], F32, tag="a")
                    nc.tensor.transpose(ps_v[:], vtmp[:], ident[:])
                    nc.vector.tensor_copy(vsT[:, pi, ii * 128:(ii + 1) * 128],
                                          ps_v[:])
            cst["vsT"] = vsT

            pools = (stream, bigp, pe_pool, res_pool, psA, psZ, dram_pool)
            for _ in range(repeat):
                for b in range(B_PER_CORE):
                    _emit_batch(nc, tc, pools, cst, b, x_d, bs_d, cheb_d, out_d)
    nc.compile()
    return nc


_RUNNER_CACHE = {}


def _make_runner(repeat=1):
    """Build the Bass program once and wrap it in a persistent jitted
    shard_map executable so repeat calls skip recompile/reload."""
    import jax
    from jax.sharding import Mesh, PartitionSpec
    from jax.experimental.shard_map import shard_map
    from concourse import bass2jax, mybir as _mybir

    nc = build_nc(repeat)
    bass2jax.install_neuronx_cc_hook()

    part_name = nc.partition_id_tensor.name if nc.partition_id_tensor else None
    in_names = []
    out_names = []
    out_avals = []
    zero_outs = []
    for alloc in nc.m.functions[0].allocations:
        if not isinstance(_mybir.MemoryLocationSet, type) or not isinstance(
                alloc, _mybir.MemoryLocationSet):
            continue
        name = alloc.memorylocations[0].name
        if alloc.kind == "ExternalInput":
            if name != part_name:
                in_names.append(name)
        elif alloc.kind == "ExternalOutput":
            out_names.append(name)
            shape = tuple(alloc.tensor_shape)
            dtype = _mybir.dt.np(alloc.dtype)
            out_avals.append(jax.core.ShapedArray(shape, dtype))
            zero_outs.append(np.zeros(shape, dtype))
    n_params = len(in_names)
    all_names = in_names + out_names
    if part_name is not None:
        all_names = all_names + [part_name]

    def _body(*args):
        operands = list(args)
        if part_name is not None:
            operands.append(bass2jax.partition_id_tensor())
        outs = bass2jax._bass_exec_p.bind(
            *operands,
            out_avals=tuple(out_avals),
            in_names=tuple(all_names),
            out_names=tuple(out_names),
            lowering_input_output_aliases=(),
            sim_require_finite=False,
            sim_require_nnan=False,
            nc=nc,
        )
        return tuple(outs)

    n_cores = 8
    devices = jax.devices()[:n_cores]
    mesh = Mesh(np.asarray(devices), ("core",))
    in_specs = (PartitionSpec("core"),) * (n_params + len(out_names))
    out_specs = (PartitionSpec("core"),) * len(out_names)
    sharded = jax.jit(
        shard_map(_body, mesh=mesh, in_specs=in_specs, out_specs=out_specs,
                  check_rep=False),
        keep_unused=True,
    )
    return nc, sharded, in_names, out_names, zero_outs, n_cores, mesh


def _get_runner(repeat=1):
    if repeat not in _RUNNER_CACHE:
        _RUNNER_CACHE[repeat] = _make_runner(repeat)
    return _RUNNER_CACHE[repeat]


def kernel(x, W1, W2, W3, bs, Vs, cheb, Theta, repeat=1):
    x = np.asarray(x, dtype=np.float32)
    full = {
        "W1": np.asarray(W1, dtype=np.float32),
        "W2": np.asarray(W2, dtype=np.float32),
        "W3": np.asarray(W3, dtype=np.float32),
        "bs": np.asarray(bs, dtype=np.float32).reshape(N, N),
        "Vs": np.asarray(Vs, dtype=np.float32),
        "cheb": np.asarray(cheb, dtype=np.float32),
        "Theta": np.asarray(Theta, dtype=np.float32),
    }
    nc, sharded, in_names, out_names, zero_outs, n_cores, mesh = _get_runner(repeat)
    # global operands concatenated over cores (shard_map splits axis 0)
    ops = []
    for name in in_names:
        if name == "x":
            ops.append(x.reshape(n_cores * B_PER_CORE, N, F, T))
        else:
            v = full[name]
            ops.append(np.concatenate([v] * n_cores, axis=0))
    for z in zero_outs:
        ops.append(np.zeros((n_cores * z.shape[0], *z.shape[1:]), z.dtype))
    out_arrs = sharded(*ops)
    out = np.asarray(out_arrs[out_names.index("out")])
    return out.reshape(16, N, FO, T)


def _staged_ops(x, full, in_names, zero_outs, n_cores):
    ops = []
    for name in in_names:
        if name == "x":
            ops.append(np.ascontiguousarray(x.reshape(n_cores * B_PER_CORE, N, F, T)))
        else:
            v = full[name]
            ops.append(np.concatenate([v] * n_cores, axis=0))
    for z in zero_outs:
        ops.append(np.zeros((n_cores * z.shape[0], *z.shape[1:]), z.dtype))
    return ops


def bench(inputs, repeat=1, iters=6):
    """Device-resident timing: returns best wall seconds per sharded() call."""
    import time as _time
    import jax
    from jax.sharding import NamedSharding, PartitionSpec
    x = np.asarray(inputs["x"], dtype=np.float32)
    full = {k: np.asarray(v, dtype=np.float32) for k, v in inputs.items() if k != "x"}
    full["bs"] = full["bs"].reshape(N, N)
    nc, sharded, in_names, out_names, zero_outs, n_cores, mesh = _get_runner(repeat)
    ops = _staged_ops(x, full, in_names, zero_outs, n_cores)
    sh = NamedSharding(mesh, PartitionSpec("core"))
    dev_ops = [jax.device_put(o, sh) for o in ops]
    r = sharded(*dev_ops)
    jax.block_until_ready(r)
    best = float("inf")
    for _ in range(iters):
        t0 = _time.time()
        r = sharded(*dev_ops)
        jax.block_until_ready(r)
        best = min(best, _time.time() - t0)
    return best


# revision 14
# speedup vs baseline: 454.8078x; 1.0356x over previous
"""MAMGCN submodule kernel for Trainium2, 8-core data-parallel over batch.

Problem (per reference):
  B=16, N=1024, F=64, T=12, K=3, F_OUT=64
  S = softmax_axis1(Vs @ sigmoid(lhs @ rhs^T + bs))
  out = relu(sum_k (cheb_k * S)^T @ x @ Theta_k)

Sharding: batch B=16 split across 8 cores (2 batches/core). All weights
replicated. Each core runs an identical Bass program on its shard.

Layout strategy per core/batch (n = destination node index, m = source):
  - product/P/S/E/A tiles keep m (or i) on partitions, n on free dim.
  - The cheb contraction uses x'-as-stationary matmuls producing
    z'[(t,f), n] transposed; Theta applied via block-diag (128,128)
    stationary; final (t,o)->(o,t)+transpose via PE transpose, with the
    softmax denominator folded in as a per-partition scale on the final
    relu copy.
  - All matmuls run in float32r (full PE rate at free>=256, ~1e-4 rounding).
"""
import numpy as np

import concourse.bass as bass
import concourse.mybir as mybir
import concourse.tile as tile
from concourse import bacc
from concourse.bass_utils import run_bass_kernel_spmd
from concourse.masks import make_identity

F32 = mybir.dt.float32
F32R = mybir.dt.float32r
AL = mybir.AluOpType
AF = mybir.ActivationFunctionType
AX = mybir.AxisListType

B_PER_CORE = 2
N = 1024
F = 64
T = 12
K = 3
FO = 64
NT = N // 128          # 8 n-tiles (128 rows each)
NQ = 4                 # n processed in quarters
HW = N // NQ // 1      # 256 free-dim per quarter
NC_TILES_PER_Q = HW // 128  # 2 c-subtiles of 128 per quarter
TQ = (T * F) // 128    # 6 (t,f)-chunks of x' (each = 2 t-values x 64 f)


def _emit_batch(nc, tc, pools, cst, b, x_d, bs_d, cheb_d, out_d):
    """Emit one batch's pipeline."""
    (stream, bigp, pe_pool, res_pool, psA, psZ, dram_pool) = pools

    # ---- Stage A: load x, reorder to x', attention row features ----
    xprime = bigp.tile([128, NT, T, F], F32R, tag="xp")
    xw1T = stream.tile([F, N], F32R, tag="xw1T", bufs=1)
    rhsBT = stream.tile([T, N], F32R, tag="rhsBT", bufs=1)
    for mi in range(NT):
        xnat = stream.tile([128, F, T], F32, tag="xnat")
        nc.sync.dma_start(out=xnat[:], in_=x_d.ap()[b, mi * 128:(mi + 1) * 128])
        # x' reorder (f,t) -> (t,f), rounded to fp32r
        nc.vector.tensor_copy(xprime[:, mi], xnat[:].rearrange("p f t -> p t f"))
        # xw1[n,f] = sum_t x*W1
        tmp = stream.tile([128, F, T], F32, tag="tmp", bufs=2)
        nc.vector.tensor_mul(tmp[:], xnat[:], cst["w1rep"][:])
        xw1_t = stream.tile([128, F], F32, tag="xw1t")
        nc.vector.tensor_reduce(out=xw1_t[:], in_=tmp[:], op=AL.add, axis=AX.X)
        # rhsB[n,t] = sum_f W3*x
        tmp2 = stream.tile([128, T, F], F32, tag="tmp2", bufs=2)
        nc.vector.tensor_mul(tmp2[:], xprime[:, mi].bitcast(F32), cst["w3rep"][:])
        rhsb_t = stream.tile([128, T], F32, tag="rhsbt")
        nc.vector.tensor_reduce(out=rhsb_t[:], in_=tmp2[:], op=AL.add, axis=AX.X)
        # transpose both to contraction-on-partitions layout
        pst64 = psA.tile([F, 128], F32, tag="a")
        nc.tensor.transpose(pst64[:], xw1_t[:], cst["ident"][:])
        nc.vector.tensor_copy(xw1T[:, mi * 128:(mi + 1) * 128], pst64[:])
        pst12 = psA.tile([T, 128], F32, tag="a")
        nc.tensor.transpose(pst12[:], rhsb_t[:], cst["ident"][:])
        nc.vector.tensor_copy(rhsBT[:, mi * 128:(mi + 1) * 128], pst12[:])

    # ---- Stage B: lhs^T = W2^T @ xw1^T  (12, N) ----
    lhsT_sb = stream.tile([T, N], F32R, tag="lhsT", bufs=1)
    for h in range(2):
        ps_l = psA.tile([T, 512], F32, tag="a")
        nc.tensor.matmul(ps_l[:], cst["w2r"][:], xw1T[:, h * 512:(h + 1) * 512],
                         start=True, stop=True)
        nc.vector.tensor_copy(lhsT_sb[:, h * 512:(h + 1) * 512], ps_l[:])

    # ---- per n-quarter pipeline ----
    for nh in range(NQ):
        HS = slice(nh * HW, (nh + 1) * HW)
        # Stage C: product + bs -> sigmoid -> P
        P_q = pe_pool.tile([128, NT, HW], F32R, tag="P")
        for ii in range(NT):
            ps_p = psA.tile([128, HW], F32, tag="a")
            nc.tensor.matmul(ps_p[:], lhsT_sb[:, ii * 128:(ii + 1) * 128],
                             rhsBT[:, HS], start=True, stop=True)
            bs_t = stream.tile([128, HW], F32, tag="bst")
            nc.sync.dma_start(out=bs_t[:], in_=bs_d.ap()[ii * 128:(ii + 1) * 128, HS])
            sgin = stream.tile([128, HW], F32, tag="sgin")
            nc.vector.tensor_add(sgin[:], ps_p[:], bs_t[:])
            nc.scalar.activation(P_q[:, ii], sgin[:], AF.Sigmoid)
        # Stage D: S = VsT^T @ P ; E = exp(S); colsum via ones-matmul
        E_q = pe_pool.tile([128, NT, HW], F32R, tag="E")
        ps_cs = psA.tile([1, HW], F32, tag="a")
        for ii in range(NT):
            ps_s = psA.tile([128, HW], F32, tag="a")
            for pi in range(NT):
                nc.tensor.matmul(ps_s[:], cst["vsT"][:, pi, ii * 128:(ii + 1) * 128],
                                 P_q[:, pi], start=(pi == 0), stop=(pi == NT - 1))
            nc.scalar.activation(E_q[:, ii], ps_s[:], AF.Exp)
            nc.tensor.matmul(ps_cs[:], cst["ones_r"][:], E_q[:, ii],
                             start=(ii == 0), stop=(ii == NT - 1))
        # softmax denominator reciprocal, then scatter to partitions
        cs_row = stream.tile([1, HW], F32, tag="cs")
        nc.vector.tensor_copy(cs_row[:], ps_cs[:])
        rc_d = dram_pool.tile([HW], F32, tag="rcd", name="rc_d")
        nc.sync.dma_start(out=rc_d.rearrange("(a b) -> a b", a=1),
                          in_=cs_row[:])
        rc_sc = stream.tile([128, NC_TILES_PER_Q], F32, tag="rcsc")
        nc.sync.dma_start(out=rc_sc[:],
                          in_=rc_d.rearrange("(c p) -> p c", p=128))
        recip_sb = stream.tile([128, NC_TILES_PER_Q], F32, tag="recip")
        nc.vector.reciprocal(recip_sb[:], rc_sc[:])
        # Stage E: A_k = cheb_k * E
        A_q = pe_pool.tile([128, K, NT, HW], F32R, tag="A", bufs=1)
        for mi in range(NT):
            for k in range(K):
                cheb_t = stream.tile([128, HW], F32, tag="chebt")
                nc.sync.dma_start(
                    out=cheb_t[:],
                    in_=cheb_d.ap()[k, mi * 128:(mi + 1) * 128, HS])
                nc.vector.tensor_mul(A_q[:, k, mi], cheb_t[:],
                                     E_q[:, mi].bitcast(F32))
        # Stage F: z' = x'^T-chunks @ A ; Theta via block-diag; transpose out
        res_tiles = []
        for _c in range(NC_TILES_PER_Q):
            res_c = res_pool.tile([128, FO, T], F32, tag="res", name=f"res{_c}")
            res_tiles.append(res_c)
        for q in range(TQ):
            ps_z = psZ.tile([128, K, HW], F32, tag="z", padded_shape=[128, K, 512])
            for mi in range(NT):
                for k in range(K):
                    nc.tensor.matmul(ps_z[:, k], xprime[:, mi, 2 * q:2 * q + 2, :],
                                     A_q[:, k, mi],
                                     start=(mi == 0), stop=(mi == NT - 1))
            ps_o = psA.tile([128, HW], F32, tag="a")
            for k in range(K):
                zs = stream.tile([128, HW], F32R, tag="zs")
                nc.vector.tensor_copy(zs[:], ps_z[:, k])
                nc.tensor.matmul(ps_o[:], cst["thbd"][:, k, :], zs[:],
                                 start=(k == 0), stop=(k == K - 1))
            os_t = stream.tile([128, HW], F32, tag="os")
            nc.vector.tensor_copy(os_t[:], ps_o[:])
            for c in range(NC_TILES_PER_Q):
                ps_tr = psA.tile([128, 128], F32, tag="a")
                nc.tensor.transpose(ps_tr[:], os_t[:, c * 128:(c + 1) * 128],
                                    cst["ident"][:])
                nc.vector.tensor_scalar(
                    out=res_tiles[c][:, :, 2 * q:2 * q + 2],
                    in0=ps_tr[:].rearrange("p (dt o) -> p o dt", o=FO),
                    scalar1=recip_sb[:, c:c + 1],
                    scalar2=0.0,
                    op0=AL.mult,
                    op1=AL.max,
                )
        for c in range(NC_TILES_PER_Q):
            nt_i = nh * NC_TILES_PER_Q + c
            nc.sync.dma_start(
                out=out_d.ap()[b, nt_i * 128:(nt_i + 1) * 128],
                in_=res_tiles[c][:])


def build_nc(repeat=1):
    nc = bacc.Bacc("TRN2", target_bir_lowering=False, debug=False, num_devices=8)
    x_d = nc.dram_tensor("x", [B_PER_CORE, N, F, T], F32, kind="ExternalInput")
    w1_d = nc.dram_tensor("W1", [T], F32, kind="ExternalInput")
    w2_d = nc.dram_tensor("W2", [F, T], F32, kind="ExternalInput")
    w3_d = nc.dram_tensor("W3", [F], F32, kind="ExternalInput")
    bs_d = nc.dram_tensor("bs", [N, N], F32, kind="ExternalInput")
    vs_d = nc.dram_tensor("Vs", [N, N], F32, kind="ExternalInput")
    cheb_d = nc.dram_tensor("cheb", [K, N, N], F32, kind="ExternalInput")
    th_d = nc.dram_tensor("Theta", [K, F, FO], F32, kind="ExternalInput")
    out_d = nc.dram_tensor("out", [B_PER_CORE, N, FO, T], F32,
                           kind="ExternalOutput")

    with tile.TileContext(nc) as tc:
        with (
            tc.tile_pool(name="consts", bufs=1) as consts,
            tc.tile_pool(name="stream", bufs=3) as stream,
            tc.tile_pool(name="bigp", bufs=1) as bigp,
            tc.tile_pool(name="pe", bufs=2) as pe_pool,
            tc.tile_pool(name="res", bufs=4) as res_pool,
            tc.tile_pool(name="dram", bufs=2, space="DRAM") as dram_pool,
            tc.tile_pool(name="psA", bufs=4, space="PSUM") as psA,
            tc.tile_pool(name="psZ", bufs=1, space="PSUM") as psZ,
        ):
            cst = {}
            ident = consts.tile([128, 128], F32)
            make_identity(nc, ident[:])
            cst["ident"] = ident
            # ones vectors (fp32r via rounding copy)
            onesf = consts.tile([128, 1], F32)
            nc.vector.memset(onesf[:], 1.0)
            ones_r = consts.tile([128, 1], F32R)
            nc.vector.tensor_copy(ones_r[:], onesf[:])
            cst["ones_r"] = ones_r
            # broadcast W1 / W3 replicas
            w1rep = consts.tile([128, F, T], F32)
            nc.gpsimd.dma_start(
                out=w1rep[:],
                in_=bass.AP(tensor=w1_d, offset=0, ap=[[0, 128], [0, F], [1, T]]))
            cst["w1rep"] = w1rep
            w3rep = consts.tile([128, T, F], F32)
            nc.gpsimd.dma_start(
                out=w3rep[:],
                in_=bass.AP(tensor=w3_d, offset=0, ap=[[0, 128], [0, T], [1, F]]))
            cst["w3rep"] = w3rep
            # W2 (f, t) fp32r
            w2f = consts.tile([F, T], F32)
            nc.sync.dma_start(out=w2f[:], in_=w2_d.ap())
            w2r = consts.tile([F, T], F32R)
            nc.vector.tensor_copy(w2r[:], w2f[:])
            cst["w2r"] = w2r
            # block-diagonal Theta (128, K, 128)
            thbd_f = consts.tile([128, K, 128], F32)
            nc.vector.memset(thbd_f[:], 0.0)
            for k in range(K):
                nc.sync.dma_start(out=thbd_f[0:F, k, 0:FO], in_=th_d.ap()[k])
                nc.sync.dma_start(out=thbd_f[F:128, k, FO:128], in_=th_d.ap()[k])
            thbd = consts.tile([128, K, 128], F32R)
            nc.vector.tensor_copy(thbd[:], thbd_f[:])
            cst["thbd"] = thbd
            # VsT (p-partitioned Vs transpose), fp32r
            vsT = consts.tile([128, NT, N], F32R)
            for pi in range(NT):
                for ii in range(NT):
                    vtmp = stream.tile([128, 128], F32, tag="vtmp", bufs=2)
                    nc.sync.dma_start(
                        out=vtmp[:],
                        in_=vs_d.ap()[ii * 128:(ii + 1) * 128,
                                      pi * 128:(pi + 1) * 128])
                    ps_v = psA.tile([128, 128], F32, tag="a")
                    nc.tensor.transpose(ps_v[:], vtmp[:], ident[:])
                    nc.vector.tensor_copy(vsT[:, pi, ii * 128:(ii + 1) * 128],
                                          ps_v[:])
            cst["vsT"] = vsT

            pools = (stream, bigp, pe_pool, res_pool, psA, psZ, dram_pool)
            for _ in range(repeat):
                for b in range(B_PER_CORE):
                    _emit_batch(nc, tc, pools, cst, b, x_d, bs_d, cheb_d, out_d)
    nc.compile()
    return nc


_RUNNER_CACHE = {}


def _make_runner(repeat=1):
    """Build the Bass program once and wrap it in a persistent jitted
    shard_map executable so repeat calls skip recompile/reload."""
    import jax
    from jax.sharding import Mesh, PartitionSpec
    from jax.experimental.shard_map import shard_map
    from concourse import bass2jax, mybir as _mybir

    nc = build_nc(repeat)
    bass2jax.install_neuronx_cc_hook()

    part_name = nc.partition_id_tensor.name if nc.partition_id_tensor else None
    in_names = []
    out_names = []
    out_avals = []
    zero_outs = []
    for alloc in nc.m.functions[0].allocations:
        if not isinstance(_mybir.MemoryLocationSet, type) or not isinstance(
                alloc, _mybir.MemoryLocationSet):
            continue
        name = alloc.memorylocations[0].name
        if alloc.kind == "ExternalInput":
            if name != part_name:
                in_names.append(name)
        elif alloc.kind == "ExternalOutput":
            out_names.append(name)
            shape = tuple(alloc.tensor_shape)
            dtype = _mybir.dt.np(alloc.dtype)
            out_avals.append(jax.core.ShapedArray(shape, dtype))
            zero_outs.append(np.zeros(shape, dtype))
    n_params = len(in_names)
    all_names = in_names + out_names
    if part_name is not None:
        all_names = all_names + [part_name]

    def _body(*args):
        operands = list(args)
        if part_name is not None:
            operands.append(bass2jax.partition_id_tensor())
        outs = bass2jax._bass_exec_p.bind(
            *operands,
            out_avals=tuple(out_avals),
            in_names=tuple(all_names),
            out_names=tuple(out_names),
            lowering_input_output_aliases=(),
            sim_require_finite=False,
            sim_require_nnan=False,
            nc=nc,
        )
        return tuple(outs)

    n_cores = 8
    devices = jax.devices()[:n_cores]
    mesh = Mesh(np.asarray(devices), ("core",))
    in_specs = tuple(
        PartitionSpec("core") if name == "x" else PartitionSpec()
        for name in in_names
    ) + (PartitionSpec("core"),) * len(out_names)
    out_specs = (PartitionSpec("core"),) * len(out_names)
    sharded = jax.jit(
        shard_map(_body, mesh=mesh, in_specs=in_specs, out_specs=out_specs,
                  check_rep=False),
        keep_unused=True,
    )
    return nc, sharded, in_names, out_names, zero_outs, n_cores, mesh


def _get_runner(repeat=1):
    if repeat not in _RUNNER_CACHE:
        _RUNNER_CACHE[repeat] = _make_runner(repeat)
    return _RUNNER_CACHE[repeat]


def kernel(x, W1, W2, W3, bs, Vs, cheb, Theta, repeat=1):
    x = np.asarray(x, dtype=np.float32)
    full = {
        "W1": np.asarray(W1, dtype=np.float32),
        "W2": np.asarray(W2, dtype=np.float32),
        "W3": np.asarray(W3, dtype=np.float32),
        "bs": np.asarray(bs, dtype=np.float32).reshape(N, N),
        "Vs": np.asarray(Vs, dtype=np.float32),
        "cheb": np.asarray(cheb, dtype=np.float32),
        "Theta": np.asarray(Theta, dtype=np.float32),
    }
    nc, sharded, in_names, out_names, zero_outs, n_cores, mesh = _get_runner(repeat)
    ops = _staged_ops(x, full, in_names, zero_outs, n_cores)
    out_arrs = sharded(*ops)
    out = np.asarray(out_arrs[out_names.index("out")])
    return out.reshape(16, N, FO, T)


def _staged_ops(x, full, in_names, zero_outs, n_cores):
    ops = []
    for name in in_names:
        if name == "x":
            ops.append(np.ascontiguousarray(x.reshape(n_cores * B_PER_CORE, N, F, T)))
        else:
            ops.append(full[name])
    for z in zero_outs:
        ops.append(np.zeros((n_cores * z.shape[0], *z.shape[1:]), z.dtype))
    return ops


def bench(inputs, repeat=1, iters=6):
    """Device-resident timing: returns best wall seconds per sharded() call."""
    import time as _time
    import jax
    from jax.sharding import NamedSharding, PartitionSpec
    x = np.asarray(inputs["x"], dtype=np.float32)
    full = {k: np.asarray(v, dtype=np.float32) for k, v in inputs.items() if k != "x"}
    full["bs"] = full["bs"].reshape(N, N)
    nc, sharded, in_names, out_names, zero_outs, n_cores, mesh = _get_runner(repeat)
    ops = _staged_ops(x, full, in_names, zero_outs, n_cores)
    sh_core = NamedSharding(mesh, PartitionSpec("core"))
    sh_rep = NamedSharding(mesh, PartitionSpec())
    shardings = [sh_core if name == "x" else sh_rep for name in in_names]
    shardings += [sh_core] * len(zero_outs)
    dev_ops = [jax.device_put(o, s_) for o, s_ in zip(ops, shardings)]
    r = sharded(*dev_ops)
    jax.block_until_ready(r)
    best = float("inf")
    for _ in range(iters):
        t0 = _time.time()
        r = sharded(*dev_ops)
        jax.block_until_ready(r)
        best = min(best, _time.time() - t0)
    return best


# revision 17
# speedup vs baseline: 467.6408x; 1.0282x over previous
"""MAMGCN submodule kernel for Trainium2, 8-core data-parallel over batch.

Problem (per reference):
  B=16, N=1024, F=64, T=12, K=3, F_OUT=64
  S = softmax_axis1(Vs @ sigmoid(lhs @ rhs^T + bs))
  out = relu(sum_k (cheb_k * S)^T @ x @ Theta_k)

Sharding: batch B=16 split across 8 cores (2 batches/core). All weights
replicated. Each core runs an identical Bass program on its shard.

Layout strategy per core/batch (n = destination node index, m = source):
  - product/P/S/E/A tiles keep m (or i) on partitions, n on free dim.
  - The cheb contraction uses x'-as-stationary matmuls producing
    z'[(t,f), n] transposed; Theta applied via block-diag (128,128)
    stationary; final (t,o)->(o,t)+transpose via PE transpose, with the
    softmax denominator folded in as a per-partition scale on the final
    relu copy.
  - All matmuls run in float32r (full PE rate at free>=256, ~1e-4 rounding).
"""
import numpy as np

import concourse.bass as bass
import concourse.mybir as mybir
import concourse.tile as tile
from concourse import bacc
from concourse.bass_utils import run_bass_kernel_spmd
from concourse.masks import make_identity

F32 = mybir.dt.float32
F32R = mybir.dt.float32r
AL = mybir.AluOpType
AF = mybir.ActivationFunctionType
AX = mybir.AxisListType

B_PER_CORE = 2
N = 1024
F = 64
T = 12
K = 3
FO = 64
NT = N // 128          # 8 n-tiles (128 rows each)
NQ = 4                 # n processed in quarters
HW = N // NQ // 1      # 256 free-dim per quarter
NC_TILES_PER_Q = HW // 128  # 2 c-subtiles of 128 per quarter
TQ = (T * F) // 128    # 6 (t,f)-chunks of x' (each = 2 t-values x 64 f)


def _emit_batch(nc, tc, pools, cst, b, x_d, bs_d, cheb_d, out_d):
    """Emit one batch's pipeline."""
    (stream, bigp, pe_pool, res_pool, psA, psZ, dram_pool) = pools

    # ---- Stage A: load x, reorder to x', attention row features ----
    xprime = bigp.tile([128, NT, T, F], F32R, tag="xp")
    xw1T = stream.tile([F, N], F32R, tag="xw1T", bufs=1)
    rhsBT = stream.tile([T, N], F32R, tag="rhsBT", bufs=1)
    for mi in range(NT):
        xnat = stream.tile([128, F, T], F32, tag="xnat", bufs=2)
        nc.sync.dma_start(out=xnat[:], in_=x_d.ap()[b, mi * 128:(mi + 1) * 128])
        # x' reorder (f,t) -> (t,f), rounded to fp32r
        nc.vector.tensor_copy(xprime[:, mi], xnat[:].rearrange("p f t -> p t f"))
        # xw1[n,f] = sum_t x*W1
        tmp = stream.tile([128, F, T], F32, tag="tmp", bufs=2)
        nc.vector.tensor_mul(tmp[:], xnat[:], cst["w1rep"][:])
        xw1_t = stream.tile([128, F], F32, tag="xw1t")
        nc.vector.tensor_reduce(out=xw1_t[:], in_=tmp[:], op=AL.add, axis=AX.X)
        # rhsB[n,t] = sum_f W3*x
        tmp2 = stream.tile([128, T, F], F32, tag="tmp2", bufs=2)
        nc.vector.tensor_mul(tmp2[:], xprime[:, mi].bitcast(F32), cst["w3rep"][:])
        rhsb_t = stream.tile([128, T], F32, tag="rhsbt")
        nc.vector.tensor_reduce(out=rhsb_t[:], in_=tmp2[:], op=AL.add, axis=AX.X)
        # transpose both to contraction-on-partitions layout
        pst64 = psA.tile([F, 128], F32, tag="a")
        nc.tensor.transpose(pst64[:], xw1_t[:], cst["ident"][:])
        nc.vector.tensor_copy(xw1T[:, mi * 128:(mi + 1) * 128], pst64[:])
        pst12 = psA.tile([T, 128], F32, tag="a")
        nc.tensor.transpose(pst12[:], rhsb_t[:], cst["ident"][:])
        nc.vector.tensor_copy(rhsBT[:, mi * 128:(mi + 1) * 128], pst12[:])

    # ---- Stage B: lhs^T = W2^T @ xw1^T  (12, N) ----
    lhsT_sb = stream.tile([T, N], F32R, tag="lhsT", bufs=1)
    for h in range(2):
        ps_l = psA.tile([T, 512], F32, tag="a")
        nc.tensor.matmul(ps_l[:], cst["w2r"][:], xw1T[:, h * 512:(h + 1) * 512],
                         start=True, stop=True)
        nc.vector.tensor_copy(lhsT_sb[:, h * 512:(h + 1) * 512], ps_l[:])

    # ---- per n-quarter pipeline ----
    for nh in range(NQ):
        HS = slice(nh * HW, (nh + 1) * HW)
        # Stage C: product + bs -> sigmoid -> P
        P_q = pe_pool.tile([128, NT, HW], F32R, tag="P")
        for ii in range(NT):
            ps_p = psA.tile([128, HW], F32, tag="a")
            nc.tensor.matmul(ps_p[:], lhsT_sb[:, ii * 128:(ii + 1) * 128],
                             rhsBT[:, HS], start=True, stop=True)
            bs_t = stream.tile([128, HW], F32, tag="bst")
            nc.sync.dma_start(out=bs_t[:], in_=bs_d.ap()[ii * 128:(ii + 1) * 128, HS])
            sgin = stream.tile([128, HW], F32, tag="sgin")
            nc.vector.tensor_add(sgin[:], ps_p[:], bs_t[:])
            nc.scalar.activation(P_q[:, ii], sgin[:], AF.Sigmoid)
        # Stage D: S = VsT^T @ P ; E = exp(S); colsum via ones-matmul
        E_q = pe_pool.tile([128, NT, HW], F32R, tag="E")
        ps_cs = psA.tile([1, HW], F32, tag="a")
        for ii in range(NT):
            ps_s = psA.tile([128, HW], F32, tag="a")
            for pi in range(NT):
                nc.tensor.matmul(ps_s[:], cst["vsT"][:, pi, ii * 128:(ii + 1) * 128],
                                 P_q[:, pi], start=(pi == 0), stop=(pi == NT - 1))
            nc.scalar.activation(E_q[:, ii], ps_s[:], AF.Exp)
            nc.tensor.matmul(ps_cs[:], cst["ones_r"][:], E_q[:, ii],
                             start=(ii == 0), stop=(ii == NT - 1))
        # softmax denominator reciprocal, then scatter to partitions
        cs_row = stream.tile([1, HW], F32, tag="cs")
        nc.vector.tensor_copy(cs_row[:], ps_cs[:])
        rc_d = dram_pool.tile([HW], F32, tag="rcd", name="rc_d")
        nc.sync.dma_start(out=rc_d.rearrange("(a b) -> a b", a=1),
                          in_=cs_row[:])
        rc_sc = stream.tile([128, NC_TILES_PER_Q], F32, tag="rcsc")
        nc.sync.dma_start(out=rc_sc[:],
                          in_=rc_d.rearrange("(c p) -> p c", p=128))
        recip_sb = stream.tile([128, NC_TILES_PER_Q], F32, tag="recip")
        nc.vector.reciprocal(recip_sb[:], rc_sc[:])
        # Stage E: A_k = cheb_k * E
        A_q = pe_pool.tile([128, K, NT, HW], F32R, tag="A", bufs=2)
        for mi in range(NT):
            for k in range(K):
                cheb_t = stream.tile([128, HW], F32, tag="chebt")
                nc.sync.dma_start(
                    out=cheb_t[:],
                    in_=cheb_d.ap()[k, mi * 128:(mi + 1) * 128, HS])
                nc.vector.tensor_mul(A_q[:, k, mi], cheb_t[:],
                                     E_q[:, mi].bitcast(F32))
        # Stage F: z' = x'^T-chunks @ A ; Theta via block-diag; transpose out
        res_tiles = []
        for _c in range(NC_TILES_PER_Q):
            res_c = res_pool.tile([128, FO, T], F32, tag="res", name=f"res{_c}")
            res_tiles.append(res_c)
        for q in range(TQ):
            ps_z = psZ.tile([128, K, HW], F32, tag="z", padded_shape=[128, K, 512])
            for mi in range(NT):
                for k in range(K):
                    nc.tensor.matmul(ps_z[:, k], xprime[:, mi, 2 * q:2 * q + 2, :],
                                     A_q[:, k, mi],
                                     start=(mi == 0), stop=(mi == NT - 1))
            ps_o = psA.tile([128, HW], F32, tag="a")
            for k in range(K):
                zs = stream.tile([128, HW], F32R, tag="zs")
                nc.scalar.copy(zs[:], ps_z[:, k])
                nc.tensor.matmul(ps_o[:], cst["thbd"][:, k, :], zs[:],
                                 start=(k == 0), stop=(k == K - 1))
            os_t = stream.tile([128, HW], F32, tag="os")
            nc.scalar.copy(os_t[:], ps_o[:])
            for c in range(NC_TILES_PER_Q):
                ps_tr = psA.tile([128, 128], F32, tag="a")
                nc.tensor.transpose(ps_tr[:], os_t[:, c * 128:(c + 1) * 128],
                                    cst["ident"][:])
                nc.vector.tensor_scalar(
                    out=res_tiles[c][:, :, 2 * q:2 * q + 2],
                    in0=ps_tr[:].rearrange("p (dt o) -> p o dt", o=FO),
                    scalar1=recip_sb[:, c:c + 1],
                    scalar2=0.0,
                    op0=AL.mult,
                    op1=AL.max,
                )
        for c in range(NC_TILES_PER_Q):
            nt_i = nh * NC_TILES_PER_Q + c
            nc.sync.dma_start(
                out=out_d.ap()[b, nt_i * 128:(nt_i + 1) * 128],
                in_=res_tiles[c][:])


def build_nc(repeat=1):
    nc = bacc.Bacc("TRN2", target_bir_lowering=False, debug=False, num_devices=8)
    x_d = nc.dram_tensor("x", [B_PER_CORE, N, F, T], F32, kind="ExternalInput")
    w1_d = nc.dram_tensor("W1", [T], F32, kind="ExternalInput")
    w2_d = nc.dram_tensor("W2", [F, T], F32, kind="ExternalInput")
    w3_d = nc.dram_tensor("W3", [F], F32, kind="ExternalInput")
    bs_d = nc.dram_tensor("bs", [N, N], F32, kind="ExternalInput")
    vs_d = nc.dram_tensor("Vs", [N, N], F32, kind="ExternalInput")
    cheb_d = nc.dram_tensor("cheb", [K, N, N], F32, kind="ExternalInput")
    th_d = nc.dram_tensor("Theta", [K, F, FO], F32, kind="ExternalInput")
    out_d = nc.dram_tensor("out", [B_PER_CORE, N, FO, T], F32,
                           kind="ExternalOutput")

    with tile.TileContext(nc) as tc:
        with (
            tc.tile_pool(name="consts", bufs=1) as consts,
            tc.tile_pool(name="stream", bufs=3) as stream,
            tc.tile_pool(name="bigp", bufs=1) as bigp,
            tc.tile_pool(name="pe", bufs=2) as pe_pool,
            tc.tile_pool(name="res", bufs=4) as res_pool,
            tc.tile_pool(name="dram", bufs=2, space="DRAM") as dram_pool,
            tc.tile_pool(name="psA", bufs=4, space="PSUM") as psA,
            tc.tile_pool(name="psZ", bufs=1, space="PSUM") as psZ,
        ):
            cst = {}
            ident = consts.tile([128, 128], F32)
            make_identity(nc, ident[:])
            cst["ident"] = ident
            # ones vectors (fp32r via rounding copy)
            onesf = consts.tile([128, 1], F32)
            nc.vector.memset(onesf[:], 1.0)
            ones_r = consts.tile([128, 1], F32R)
            nc.vector.tensor_copy(ones_r[:], onesf[:])
            cst["ones_r"] = ones_r
            # broadcast W1 / W3 replicas
            w1rep = consts.tile([128, F, T], F32)
            nc.gpsimd.dma_start(
                out=w1rep[:],
                in_=bass.AP(tensor=w1_d, offset=0, ap=[[0, 128], [0, F], [1, T]]))
            cst["w1rep"] = w1rep
            w3rep = consts.tile([128, T, F], F32)
            nc.gpsimd.dma_start(
                out=w3rep[:],
                in_=bass.AP(tensor=w3_d, offset=0, ap=[[0, 128], [0, T], [1, F]]))
            cst["w3rep"] = w3rep
            # W2 (f, t) fp32r
            w2f = consts.tile([F, T], F32)
            nc.sync.dma_start(out=w2f[:], in_=w2_d.ap())
            w2r = consts.tile([F, T], F32R)
            nc.vector.tensor_copy(w2r[:], w2f[:])
            cst["w2r"] = w2r
            # block-diagonal Theta (128, K, 128)
            thbd_f = consts.tile([128, K, 128], F32)
            nc.vector.memset(thbd_f[:], 0.0)
            for k in range(K):
                nc.sync.dma_start(out=thbd_f[0:F, k, 0:FO], in_=th_d.ap()[k])
                nc.sync.dma_start(out=thbd_f[F:128, k, FO:128], in_=th_d.ap()[k])
            thbd = consts.tile([128, K, 128], F32R)
            nc.vector.tensor_copy(thbd[:], thbd_f[:])
            cst["thbd"] = thbd
            # VsT (p-partitioned Vs transpose), fp32r
            vsT = consts.tile([128, NT, N], F32R)
            for pi in range(NT):
                for ii in range(NT):
                    vtmp = stream.tile([128, 128], F32, tag="vtmp", bufs=2)
                    nc.sync.dma_start(
                        out=vtmp[:],
                        in_=vs_d.ap()[ii * 128:(ii + 1) * 128,
                                      pi * 128:(pi + 1) * 128])
                    ps_v = psA.tile([128, 128], F32, tag="a")
                    nc.tensor.transpose(ps_v[:], vtmp[:], ident[:])
                    nc.vector.tensor_copy(vsT[:, pi, ii * 128:(ii + 1) * 128],
                                          ps_v[:])
            cst["vsT"] = vsT

            pools = (stream, bigp, pe_pool, res_pool, psA, psZ, dram_pool)
            for _ in range(repeat):
                for b in range(B_PER_CORE):
                    _emit_batch(nc, tc, pools, cst, b, x_d, bs_d, cheb_d, out_d)
    nc.compile()
    return nc


_RUNNER_CACHE = {}


def _make_runner(repeat=1):
    """Build the Bass program once and wrap it in a persistent jitted
    shard_map executable so repeat calls skip recompile/reload."""
    import jax
    from jax.sharding import Mesh, PartitionSpec
    from jax.experimental.shard_map import shard_map
    from concourse import bass2jax, mybir as _mybir

    nc = build_nc(repeat)
    bass2jax.install_neuronx_cc_hook()

    part_name = nc.partition_id_tensor.name if nc.partition_id_tensor else None
    in_names = []
    out_names = []
    out_avals = []
    zero_outs = []
    for alloc in nc.m.functions[0].allocations:
        if not isinstance(_mybir.MemoryLocationSet, type) or not isinstance(
                alloc, _mybir.MemoryLocationSet):
            continue
        name = alloc.memorylocations[0].name
        if alloc.kind == "ExternalInput":
            if name != part_name:
                in_names.append(name)
        elif alloc.kind == "ExternalOutput":
            out_names.append(name)
            shape = tuple(alloc.tensor_shape)
            dtype = _mybir.dt.np(alloc.dtype)
            out_avals.append(jax.core.ShapedArray(shape, dtype))
            zero_outs.append(np.zeros(shape, dtype))
    n_params = len(in_names)
    all_names = in_names + out_names
    if part_name is not None:
        all_names = all_names + [part_name]

    def _body(*args):
        operands = list(args)
        if part_name is not None:
            operands.append(bass2jax.partition_id_tensor())
        outs = bass2jax._bass_exec_p.bind(
            *operands,
            out_avals=tuple(out_avals),
            in_names=tuple(all_names),
            out_names=tuple(out_names),
            lowering_input_output_aliases=(),
            sim_require_finite=False,
            sim_require_nnan=False,
            nc=nc,
        )
        return tuple(outs)

    n_cores = 8
    devices = jax.devices()[:n_cores]
    mesh = Mesh(np.asarray(devices), ("core",))
    in_specs = tuple(
        PartitionSpec("core") if name == "x" else PartitionSpec()
        for name in in_names
    ) + (PartitionSpec("core"),) * len(out_names)
    out_specs = (PartitionSpec("core"),) * len(out_names)
    sharded = jax.jit(
        shard_map(_body, mesh=mesh, in_specs=in_specs, out_specs=out_specs,
                  check_rep=False),
        keep_unused=True,
    )
    return nc, sharded, in_names, out_names, zero_outs, n_cores, mesh


def _get_runner(repeat=1):
    if repeat not in _RUNNER_CACHE:
        _RUNNER_CACHE[repeat] = _make_runner(repeat)
    return _RUNNER_CACHE[repeat]


def kernel(x, W1, W2, W3, bs, Vs, cheb, Theta, repeat=1):
    x = np.asarray(x, dtype=np.float32)
    full = {
        "W1": np.asarray(W1, dtype=np.float32),
        "W2": np.asarray(W2, dtype=np.float32),
        "W3": np.asarray(W3, dtype=np.float32),
        "bs": np.asarray(bs, dtype=np.float32).reshape(N, N),
        "Vs": np.asarray(Vs, dtype=np.float32),
        "cheb": np.asarray(cheb, dtype=np.float32),
        "Theta": np.asarray(Theta, dtype=np.float32),
    }
    nc, sharded, in_names, out_names, zero_outs, n_cores, mesh = _get_runner(repeat)
    ops = _staged_ops(x, full, in_names, zero_outs, n_cores)
    out_arrs = sharded(*ops)
    out = np.asarray(out_arrs[out_names.index("out")])
    return out.reshape(16, N, FO, T)


def _staged_ops(x, full, in_names, zero_outs, n_cores):
    ops = []
    for name in in_names:
        if name == "x":
            ops.append(np.ascontiguousarray(x.reshape(n_cores * B_PER_CORE, N, F, T)))
        else:
            ops.append(full[name])
    for z in zero_outs:
        ops.append(np.zeros((n_cores * z.shape[0], *z.shape[1:]), z.dtype))
    return ops


def _bench_setup(inputs, repeat):
    import jax
    from jax.sharding import NamedSharding, PartitionSpec
    x = np.asarray(inputs["x"], dtype=np.float32)
    full = {k: np.asarray(v, dtype=np.float32) for k, v in inputs.items() if k != "x"}
    full["bs"] = full["bs"].reshape(N, N)
    nc, sharded, in_names, out_names, zero_outs, n_cores, mesh = _get_runner(repeat)
    ops = _staged_ops(x, full, in_names, zero_outs, n_cores)
    sh_core = NamedSharding(mesh, PartitionSpec("core"))
    sh_rep = NamedSharding(mesh, PartitionSpec())
    shardings = [sh_core if name == "x" else sh_rep for name in in_names]
    shardings += [sh_core] * len(zero_outs)
    dev_ops = [jax.device_put(o, s_) for o, s_ in zip(ops, shardings)]
    jax.block_until_ready(sharded(*dev_ops))
    return sharded, dev_ops


def bench_pair(inputs, rep_a=1, rep_b=9, iters=20):
    """Interleaved device-resident timing of two repeat variants.
    Returns (best_a, best_b) seconds — interleaving cancels slow drift in the
    fixed dispatch overhead."""
    import time as _time
    import jax
    sh_a, ops_a = _bench_setup(inputs, rep_a)
    sh_b, ops_b = _bench_setup(inputs, rep_b)
    best_a = best_b = float("inf")
    for _ in range(iters):
        t0 = _time.time()
        jax.block_until_ready(sh_a(*ops_a))
        best_a = min(best_a, _time.time() - t0)
        t0 = _time.time()
        jax.block_until_ready(sh_b(*ops_b))
        best_b = min(best_b, _time.time() - t0)
    return best_a, best_b
